# revision 20
# baseline (speedup 1.0000x reference)
"""Trainium2 Bass kernel for nn_BertHungarianLoss.

Reference computation (M=8, V=128000, P=8!=40320):
    prob  = softmax(logits)                              [M, V]
    score[p] = sum_j prob[j, target[perms[p, j]]]        [P]
    best  = argmax(score)  (first max)
    tb    = target[perms[best]]                          [M]
    loss  = -log_softmax(logits)[j, tb[j]]               [M]
    returns (loss, tb)

Distribution over 8 NeuronCores:
  - vocab-parallel softmax denominators: core k reduces exp() over
    logits[:, 16000k:16000(k+1)]; partials combined with AllGather #1.
  - permutation-parallel scoring: core k scores perms [5040k, 5040(k+1))
    via a one-hot/PE-matmul formulation; per-core winners are combined
    (score-max, first-index tiebreak) with AllGather #2; each core then
    selects the winning candidate's loss/target vectors.

All compute (softmax stats, gather of target logits, permutation scoring,
argmax, CE loss) happens on device; the host only slices/stages inputs and
reads core 0's output.
"""

import numpy as np

import concourse.bacc as bacc
import concourse.bass as bass
import concourse.mybir as mybir
import concourse.tile as tile
from concourse.bass import IndirectOffsetOnAxis
from concourse.bass_utils import run_bass_kernel_spmd

M = 8
V = 128000
P = 40320            # 8!
NCORES = 8
VSL = V // NCORES    # 16000 vocab slice
PSL = P // NCORES    # 5040 perms per core
HALF = PSL // 2      # 2520 (two perms K-packed per matmul column)
NMM = 5              # matmuls of 504 columns each
NCOL = HALF // NMM   # 504

f32 = mybir.dt.float32
i32 = mybir.dt.int32
u8 = mybir.dt.uint8

AF = mybir.ActivationFunctionType
OP = mybir.AluOpType
AX = mybir.AxisListType

BIG = 1.0e9


def build_program(dbg=False, sim=False):
    nc = bacc.Bacc("TRN2", target_bir_lowering=False, debug=False,
                   num_devices=NCORES)

    # ---- I/O ----
    lg = nc.dram_tensor("lg", [M, VSL], f32, kind="ExternalInput").ap()
    lgf = nc.dram_tensor("lgf", [M, V], f32, kind="ExternalInput").ap()
    tgt = nc.dram_tensor("tgt", [1, M], i32, kind="ExternalInput").ap()
    # indirect-DMA source view: flat [N, 1], coef=1; offsets are element
    # indices, one offset per output partition (HW-verified semantics).
    lgf_ind = lgf.rearrange("j v -> (j v)").unsqueeze(1)
    pv = nc.dram_tensor("pv", [128, HALF], u8, kind="ExternalInput").ap()
    pml = nc.dram_tensor("pml", [PSL, M], u8, kind="ExternalInput").ap()
    ivec = nc.dram_tensor("ivec", [128, 1], f32, kind="ExternalInput").ap()
    blk = nc.dram_tensor("blk", [128, 2], f32, kind="ExternalInput").ap()
    pidx = nc.dram_tensor("pidx", [126, 40], f32, kind="ExternalInput").ap()
    ob = nc.dram_tensor("ob", [64, 1], i32, kind="ExternalInput").ap()
    iv64 = nc.dram_tensor("iv64", [1, 64], f32, kind="ExternalInput").ap()
    io8k = nc.dram_tensor("io8k", [1, M], f32, kind="ExternalInput").ap()
    o_loss = nc.dram_tensor("loss", [1, M], f32, kind="ExternalOutput").ap()
    o_tb = nc.dram_tensor("tbest", [1, M], i32, kind="ExternalOutput").ap()

    rg = [list(range(NCORES))]

    with tile.TileContext(nc) as tc:
        with tc.tile_pool(name="sb", bufs=1) as sb, \
             tc.tile_pool(name="dr", bufs=1, space="DRAM") as dr, \
             tc.tile_pool(name="ps", bufs=1, space="PSUM") as ps:

            # ---------- stage in ----------
            L = sb.tile([128, VSL // 128 * 8], f32)       # [128, 1000]
            nc.sync.dma_start(L[:], lg.rearrange("j (s c) -> (j s) c", s=16))
            pv_t = sb.tile([128, HALF], u8)
            nc.sync.dma_start(pv_t[:], pv)
            ivec_t = sb.tile([128, 1], f32)
            nc.sync.dma_start(ivec_t[:], ivec)
            blk_t = sb.tile([128, 2], f32)
            nc.sync.dma_start(blk_t[:], blk)
            pidx_t = sb.tile([126, 40], f32)
            nc.sync.dma_start(pidx_t[:], pidx)
            ob_t = sb.tile([1, 64], i32)
            nc.sync.dma_start(ob_t[:], ob)
            iv64_t = sb.tile([1, 64], f32)
            nc.sync.dma_start(iv64_t[:], iv64)
            io8k_t = sb.tile([1, M], f32)
            nc.sync.dma_start(io8k_t[:], io8k)
            tgt_t = sb.tile([1, M], i32)
            nc.sync.dma_start(tgt_t[:], tgt)

            # ---------- softmax partials ----------
            E = sb.tile([128, VSL // 128 * 8], f32)       # exp scratch
            sums = sb.tile([128, 1], f32)
            nc.scalar.activation(E[:], L[:], AF.Exp, accum_out=sums[:])

            # ---------- gather logits at target columns ----------
            # r-layout: r = i*8 + j (j fastest). Indirect DMA gathers one
            # element per output PARTITION (offset r -> partition r).
            t64row = sb.tile([1, 64], i32)
            nc.vector.tensor_copy(
                t64row[:].rearrange("p (i j) -> p i j", j=8),
                tgt_t[:].unsqueeze(2).to_broadcast((1, 8, 8)))
            d_t64 = dr.tile([1, 64], i32)
            nc.sync.dma_start(d_t64[:], t64row[:])
            tgt_p = sb.tile([64, 1], i32)
            nc.sync.dma_start(tgt_p[:],
                              d_t64[:].rearrange("a b -> (a b)").unsqueeze(1))
            ob_p = sb.tile([64, 1], i32)
            nc.sync.dma_start(ob_p[:], ob)
            offs_p = sb.tile([64, 1], i32)
            nc.vector.tensor_tensor(offs_p[:], ob_p[:], tgt_p[:], OP.add)
            T_p = sb.tile([64, 1], f32)
            nc.gpsimd.indirect_dma_start(
                T_p[:], None, lgf_ind,
                IndirectOffsetOnAxis(ap=offs_p[:], axis=0))
            expT_p = sb.tile([64, 1], f32)
            nc.scalar.activation(expT_p[:], T_p[:], AF.Exp)
            # row form of T for the candidate phase
            d_T64 = dr.tile([64, 1], f32)
            nc.sync.dma_start(d_T64[:], T_p[:])
            Trow = sb.tile([1, 64], f32)
            nc.sync.dma_start(Trow[:], d_T64[:].rearrange("a b -> b a"))
            # replicate exp(T) across partitions: [64,1] -> [2,64] -> [128,1]
            d_e64 = dr.tile([64, 1], f32)
            nc.sync.dma_start(d_e64[:], expT_p[:])
            d2e = dr.tile([2, 64], f32)
            nc.sync.dma_start(d2e[:],
                              d_e64[:].rearrange("a b -> b a").to_broadcast((2, 64)))
            expT2 = sb.tile([128, 1], f32)
            nc.sync.dma_start(expT2[:],
                              d2e[:].rearrange("a b -> (a b)").unsqueeze(1))

            # ---------- AllGather #1: softmax partial sums ----------
            cc1_in = dr.tile([128, 1], f32)
            cc1_out = dr.tile([128 * NCORES, 1], f32)
            nc.sync.dma_start(cc1_in[:], sums[:])
            nc.gpsimd.collective_compute(
                "AllGather", OP.bypass, replica_groups=rg,
                ins=[cc1_in.opt()], outs=[cc1_out.opt()])
            back1 = sb.tile([1, 128 * NCORES], f32)
            nc.sync.dma_start(back1[:], cc1_out[:].rearrange("a b -> b a"))
            S8 = sb.tile([1, M], f32)
            nc.vector.tensor_reduce(
                S8[:], back1[:].rearrange("p (r j s) -> p j r s", r=8, j=8, s=16),
                axis=AX.XY, op=OP.add)
            recipS = sb.tile([1, M], f32)
            nc.vector.reciprocal(recipS[:], S8[:])
            lseN = sb.tile([1, M], f32)
            nc.scalar.activation(lseN[:], S8[:], AF.Ln)

            # replicate 1/S across partitions: r128row[c] = 1/S_{j(c)},
            # j(c) = c % 8  ->  16x tile of the 8-vector
            r128row = sb.tile([1, 128], f32)
            nc.vector.tensor_copy(
                r128row[:].rearrange("p (q j) -> p q j", q=16),
                recipS[:].unsqueeze(1).to_broadcast((1, 16, 8)))
            d_r128 = dr.tile([1, 128], f32)
            nc.sync.dma_start(d_r128[:], r128row[:])
            rec2 = sb.tile([128, 1], f32)
            nc.sync.dma_start(rec2[:],
                              d_r128[:].rearrange("a b -> (a b)").unsqueeze(1))

            # ---------- permutation scoring ----------
            # mw[c, m] = (pv[c, m] == i(c)) * exp(T[j(c), i(c)])   (no S dep)
            mw = sb.tile([128, HALF], f32)
            nc.vector.tensor_scalar(mw[:], pv_t[:], ivec_t[:], expT2[:],
                                    OP.is_equal, OP.mult)
            # lhsT[c, h] = (c//64 == h) / S_j(c)
            lhsT = sb.tile([128, 2], f32)
            nc.vector.tensor_scalar(lhsT[:], blk_t[:], rec2[:], None, OP.mult)

            s2 = sb.tile([2, HALF], f32)
            for u in range(NMM):
                pm = ps.tile([2, NCOL], f32, tag=f"pm{u}")
                nc.tensor.matmul(pm[:], lhsT[:], mw[:, u * NCOL:(u + 1) * NCOL],
                                 start=True, stop=True)
                if u % 2 == 0:
                    nc.vector.tensor_copy(s2[:, u * NCOL:(u + 1) * NCOL], pm[:])
                else:
                    nc.scalar.copy(s2[:, u * NCOL:(u + 1) * NCOL], pm[:])

            # repartition [2, 2520] -> [126, 40] (both enumerate local p)
            sc = sb.tile([126, 40], f32)
            nc.sync.dma_start(sc[:], s2[:])

            # ---------- local argmax (first-max) ----------
            pack = sb.tile([126, 2], f32)
            nc.vector.tensor_reduce(pack[:, 0:1], sc[:], axis=AX.X, op=OP.max)
            e1 = sb.tile([126, 40], f32)
            nc.vector.tensor_scalar(e1[:], sc[:], pack[:, 0:1], BIG,
                                    OP.is_lt, OP.mult)
            e2 = sb.tile([126, 40], f32)
            nc.vector.tensor_tensor(e2[:], e1[:], pidx_t[:], OP.add)
            nc.vector.tensor_reduce(pack[:, 1:2], e2[:], axis=AX.X, op=OP.min)

            d_c0 = dr.tile([126, 1], f32)
            d_c1 = dr.tile([126, 1], f32)
            nc.sync.dma_start(d_c0[:], pack[:, 0:1])
            nc.sync.dma_start(d_c1[:], pack[:, 1:2])
            fin2 = sb.tile([1, 252], f32)
            nc.sync.dma_start(fin2[:, 0:126], d_c0[:].rearrange("a b -> b a"))
            nc.sync.dma_start(fin2[:, 126:252], d_c1[:].rearrange("a b -> b a"))
            m_loc = sb.tile([1, 1], f32)
            nc.vector.tensor_reduce(m_loc[:], fin2[:, 0:126], axis=AX.X, op=OP.max)
            g1 = sb.tile([1, 126], f32)
            nc.vector.tensor_scalar(g1[:], fin2[:, 0:126], m_loc[:], BIG,
                                    OP.is_lt, OP.mult)
            g2 = sb.tile([1, 126], f32)
            nc.vector.tensor_tensor(g2[:], g1[:], fin2[:, 126:252], OP.add)
            i_loc = sb.tile([1, 1], f32)
            nc.vector.tensor_reduce(i_loc[:], g2[:], axis=AX.X, op=OP.min)

            # ---------- local candidate: loss/tb for this core's winner ----------
            i_loc8 = sb.tile([1, 1], f32)
            nc.vector.tensor_scalar(i_loc8[:], i_loc[:], 8.0, None, OP.mult)
            pb_offf = sb.tile([1, M], f32)
            nc.vector.tensor_scalar(pb_offf[:], io8k_t[:], i_loc8[:], None, OP.add)
            pb_offi = sb.tile([1, M], i32)
            nc.vector.tensor_copy(pb_offi[:], pb_offf[:])
            pbrow = sb.tile([1, M], u8)
            nc.gpsimd.indirect_dma_start(
                pbrow[:], None, pml.rearrange("a b -> (a b)").unsqueeze(1),
                IndirectOffsetOnAxis(ap=pb_offi[:], axis=0))
            pbf = sb.tile([1, M], f32)
            nc.vector.tensor_copy(pbf[:], pbrow[:])

            # r = i*8 + j: mask[r] = (i(r) == perm_best[j(r)])
            mask = sb.tile([1, 64], f32)
            nc.vector.tensor_tensor(
                mask[:].rearrange("p (i j) -> p i j", j=8),
                iv64_t[:].rearrange("p (i j) -> p i j", j=8),
                pbf[:].unsqueeze(1).to_broadcast((1, 8, 8)), OP.is_equal)
            tm = sb.tile([1, 64], f32)
            nc.vector.tensor_tensor(tm[:], mask[:], Trow[:], OP.mult)
            Tb = sb.tile([1, M], f32)
            nc.vector.tensor_reduce(Tb[:],
                                    tm[:].rearrange("p (i j) -> p j i", j=8),
                                    axis=AX.X, op=OP.add)
            lcand = sb.tile([1, M], f32)
            nc.vector.tensor_tensor(lcand[:], lseN[:], Tb[:], OP.subtract)

            tgf = sb.tile([1, M], f32)
            nc.vector.tensor_copy(tgf[:], tgt_t[:])
            tm2 = sb.tile([1, 64], f32)
            nc.vector.tensor_tensor(
                tm2[:].rearrange("p (i j) -> p i j", j=8),
                mask[:].rearrange("p (i j) -> p i j", j=8),
                tgf[:].unsqueeze(2).to_broadcast((1, 8, 8)), OP.mult)
            tbc = sb.tile([1, M], f32)
            nc.vector.tensor_reduce(tbc[:],
                                    tm2[:].rearrange("p (i j) -> p j i", j=8),
                                    axis=AX.X, op=OP.add)

            cand = sb.tile([1, 24], f32)
            nc.vector.memset(cand[:], 0.0)
            nc.vector.tensor_copy(cand[:, 0:1], m_loc[:])
            nc.vector.tensor_copy(cand[:, 1:2], i_loc[:])
            nc.vector.tensor_copy(cand[:, 2:10], lcand[:])
            nc.vector.tensor_copy(cand[:, 10:18], tbc[:])

            # ---------- AllGather #2: candidates ----------
            cc2_in = dr.tile([1, 24], f32)
            cc2_out = dr.tile([NCORES, 24], f32)
            nc.sync.dma_start(cc2_in[:], cand[:])
            nc.gpsimd.collective_compute(
                "AllGather", OP.bypass, replica_groups=rg,
                ins=[cc2_in.opt()], outs=[cc2_out.opt()])
            back2 = sb.tile([1, NCORES * 24], f32)
            nc.sync.dma_start(back2[:].rearrange("p (r c) -> p r c", r=NCORES),
                              cc2_out[:])
            b2 = back2[:].rearrange("p (r c) -> p r c", r=NCORES)
            scr = b2[:, :, 0]               # [1, 8] stride 24
            idxr = b2[:, :, 1]
            loss_all = b2[:, :, 2:10].transpose([0, 2, 1])   # [1, 8j, 8r]
            tb_all = b2[:, :, 10:18].transpose([0, 2, 1])

            m_fin = sb.tile([1, 1], f32)
            nc.vector.tensor_reduce(m_fin[:], scr, axis=AX.X, op=OP.max)
            f1 = sb.tile([1, NCORES], f32)
            nc.vector.tensor_scalar(f1[:], scr, m_fin[:], BIG, OP.is_lt, OP.mult)
            f2 = sb.tile([1, NCORES], f32)
            nc.vector.tensor_tensor(f2[:], f1[:], idxr, OP.add)
            i_fin = sb.tile([1, 1], f32)
            nc.vector.tensor_reduce(i_fin[:], f2[:], axis=AX.X, op=OP.min)
            sel = sb.tile([1, NCORES], f32)
            nc.vector.tensor_scalar(sel[:], f2[:], i_fin[:], None, OP.is_equal)

            lsel = sb.tile([1, 64], f32)
            nc.vector.tensor_tensor(
                lsel[:].rearrange("p (j r) -> p j r", r=8), loss_all,
                sel[:].unsqueeze(1).to_broadcast((1, 8, 8)), OP.mult)
            lossF = sb.tile([1, M], f32)
            nc.vector.tensor_reduce(lossF[:],
                                    lsel[:].rearrange("p (j r) -> p j r", r=8),
                                    axis=AX.X, op=OP.add)
            tsel = sb.tile([1, 64], f32)
            nc.vector.tensor_tensor(
                tsel[:].rearrange("p (j r) -> p j r", r=8), tb_all,
                sel[:].unsqueeze(1).to_broadcast((1, 8, 8)), OP.mult)
            tbFf = sb.tile([1, M], f32)
            nc.vector.tensor_reduce(tbFf[:],
                                    tsel[:].rearrange("p (j r) -> p j r", r=8),
                                    axis=AX.X, op=OP.add)
            tbFi = sb.tile([1, M], i32)
            nc.vector.tensor_copy(tbFi[:], tbFf[:])

            nc.sync.dma_start(o_loss, lossF[:])
            nc.sync.dma_start(o_tb, tbFi[:])

            if dbg:
                def dump(name, t, shape):
                    o = nc.dram_tensor(name, shape, t.dtype,
                                       kind="ExternalOutput").ap()
                    nc.sync.dma_start(o, t)
                dump("d_sums", sums[:], [128, 1])
                dump("d_S8", S8[:], [1, M])
                dump("d_Trow", Trow[:], [1, 64])
                dump("d_expT2", expT2[:], [128, 1])
                dump("d_rec2", rec2[:], [128, 1])
                dump("d_mw", mw[:], [128, HALF])
                dump("d_lhsT", lhsT[:], [128, 2])
                dump("d_s2", s2[:], [2, HALF])
                dump("d_sc", sc[:], [126, 40])
                dump("d_pack", pack[:], [126, 2])
                dump("d_fin2", fin2[:], [1, 252])
                dump("d_iloc", i_loc[:], [1, 1])
                dump("d_mask", mask[:], [1, 64])
                dump("d_cand", cand[:], [1, 24])
                dump("d_back2", back2[:], [1, NCORES * 24])
                dump("d_lse", lseN[:], [1, M])

    nc.compile()
    return nc


_NC_CACHE = None


def _get_program():
    global _NC_CACHE
    if _NC_CACHE is None:
        _NC_CACHE = build_program()
    return _NC_CACHE


def make_in_maps(logits, target, perms):
    logits = np.ascontiguousarray(np.asarray(logits, dtype=np.float32))
    target = np.asarray(target).astype(np.int32).reshape(1, M)
    perms = np.asarray(perms).astype(np.int64)

    # r = i*8 + j convention: j(c) = c % 8, i(c) = (c % 64) // 8
    ivec = ((np.arange(128) % 64) // 8).astype(np.float32).reshape(128, 1)
    blk = np.zeros((128, 2), dtype=np.float32)
    blk[:64, 0] = 1.0
    blk[64:, 1] = 1.0
    ob = ((np.arange(64) % 8) * V).astype(np.int32).reshape(64, 1)
    iv64 = (np.arange(64) // 8).astype(np.float32).reshape(1, 64)

    c = np.arange(128)
    jc = c % 8                  # j(c)
    in_maps = []
    for k in range(NCORES):
        psl = perms[k * PSL:(k + 1) * PSL]              # [5040, 8]
        # pv[c, m] = perms_local[(c//64)*2520 + m, j(c)]
        half = (c // 64)
        pvk = psl[(half[:, None] * HALF + np.arange(HALF)[None, :]), jc[:, None]]
        in_maps.append({
            "lg": np.ascontiguousarray(logits[:, k * VSL:(k + 1) * VSL]),
            "lgf": logits,
            "tgt": target,
            "pv": pvk.astype(np.uint8),
            "pml": psl.astype(np.uint8),
            "ivec": ivec,
            "blk": blk,
            "pidx": (k * PSL + np.arange(PSL)).astype(np.float32).reshape(126, 40),
            "ob": ob,
            "iv64": iv64,
            "io8k": (np.arange(8) - 8.0 * k * PSL).astype(np.float32).reshape(1, 8),
        })
    return in_maps


def run(logits, target, perms, trace=False):
    nc = _get_program()
    in_maps = make_in_maps(logits, target, perms)
    res = run_bass_kernel_spmd(nc, in_maps, core_ids=list(range(NCORES)),
                               trace=trace)
    loss = res.results[0]["loss"].reshape(M).astype(np.float32)
    tb = res.results[0]["tbest"].reshape(M).astype(np.int32)
    return loss, tb, res


def kernel(logits, target, perms):
    loss, tb, _ = run(logits, target, perms, trace=False)
    return loss, tb
